# revision 9
# baseline (speedup 1.0000x reference)
"""Trainium2 Bass kernel for nn_KnowledgeFusion.

Math (b=8, H=W=32, d=o=256, n_obj=15):
  embs_aug = concat([embs, mean(embs)])                  [b,16,256]
  mask     = rasterized boxes (rounded to PATCH_SIZE=2)  [b,16,1024] in {0,1}
  proj     = patches @ Wp                                [b,1024,256]
  inj      = embs_aug @ We                               [b,16,256]
  s[hw]    = sum_n mask[n,hw]   (>=1: image box row)
  out      = proj + (mask^T @ inj) / s[:,None]           [b,1024,256]

The mean-emb row folds away: with inj_k = embs_k @ We (k<15),
  outT[o,hw] = Wp^T @ patchesT + inj^T @ ((mask + 1/15) * recB)
where recB = 1/s, s integer in 1..16, recovered exactly via a one-hot
(is_equal against a per-partition constant) collapsed by a tiny matmul
against weights 1/(p+1).  (DVE reciprocal measures ~6.5 cycles/elem,
so the one-hot trick stays.)

v5 layout: the pixel halves h=0/h=1 live on partition strips 0:16 and
32:48.  Because DVE cost scales with free-size only (not partitions)
and matmul cost with streamed columns only, the whole chain is FUSED
across both halves: ONE DVE op each for mask / is_eq / maskN on
[48,512], and ONE 512-column matmul each for s and recB using [48,64]
block weights (rows 16:31 flow zeros).  The strip split is kept so the
four bank-closing inj matmuls (weights on rows 0:15 vs 32:47) still
pair up concurrently on the PE array's row strips.

The PATCH_SIZE box rounding is folded into parity-precomputed grids
(grid_lo[c] = c|1 >= start  <=>  c >= start&~1;  grid_hi2[c] =
(c&~1)-1 < end  <=>  c < end + 2-(end%2)), so raw fp32 locations feed
the comparisons directly -- the Vector queue is just 4 small compare
ops, one fused mask op, is_eq and maskN.  All constants ship
pre-computed in a tiny bf16 `cg` tensor; locations (+ the is_eq
per-partition constant) ship as fp32 `locf`.  Each dma_start costs
~0.6-0.8us of sequencer issue time and completion semaphores lag the
data by ~1us, so inputs use 6 triggers ordered by first use and
outputs 2 contiguous [128,1024] tensors, one per PSUM bank pair; each
pair evacuates on ACT+DVE concurrently the moment its banks close,
overlapped with remaining PE work.  The PE stream is ordered to never
stall so its p-state stays warm.  Everything is bf16 (fp32 PSUM
accumulation); rel-err ~4.6e-3 against the 2e-2 gate.
"""

import sys

sys.path.insert(0, "/opt/trn_rl_repo")

import numpy as np

import concourse.bass as bass
import concourse.bacc as bacc
import concourse.mybir as mybir
from concourse import tile
from concourse import bass_utils
from concourse.alu_op_type import AluOpType

B, H, W, D = 8, 32, 32, 256
NOBJ, N = 15, 16
HW = H * W
O = 256
P2 = 32  # partition offset of the h=1 group
NP = 48
FP = mybir.dt.float32
BF = mybir.dt.bfloat16
AF = mybir.ActivationFunctionType

# weights blob columns (bf16): We0 We1 eTW0 eTW1 | Wp0 Wp1
WB_A = 2 * O + 2 * 64  # 640: first DMA chunk (inj_pre inputs)
WB = WB_A + 2 * O  # 1152 total (second DMA carries Wp)

# const-gadget columns (bf16): grid_lo grid_hi2 yglo yghi2 W_s W_r
G_GH = 32
G_YL = 64
G_YH = 80
G_WS = 96
G_WR = G_WS + 64
CG = G_WR + 64  # 224


def _ap(ap, free_dims):
    """AP with explicit free-dim [step, count] pairs (step 0 = broadcast)."""
    return bass.AP(ap.tensor, ap.offset, ap.ap[:1] + free_dims)


def build_nc(debug: bool = False):
    nc = bacc.Bacc("TRN2", target_bir_lowering=False, debug=debug, num_devices=B)

    locf = nc.dram_tensor("locf", [NP, 6], FP, kind="ExternalInput")
    cg = nc.dram_tensor("cg", [NP, CG], BF, kind="ExternalInput")
    wb = nc.dram_tensor("wb", [128, WB], BF, kind="ExternalInput")
    pT = nc.dram_tensor("pT", [128, 2 * HW], BF, kind="ExternalInput")
    out1 = nc.dram_tensor("out1", [128, HW], BF, kind="ExternalOutput")
    out2 = nc.dram_tensor("out2", [128, HW], BF, kind="ExternalOutput")

    with tile.TileContext(nc) as tc:
        with (
            nc.allow_low_precision(reason="bf16 matmuls, fp32 PSUM accumulation"),
            tc.tile_pool(name="big", bufs=1) as big,
            tc.tile_pool(name="small", bufs=1) as small,
            tc.tile_pool(name="outp", bufs=1) as outp,
            tc.tile_pool(name="psT", bufs=1, space=bass.MemorySpace.PSUM) as psT,
            tc.tile_pool(name="psS", bufs=1, space=bass.MemorySpace.PSUM) as psS,
            tc.tile_pool(name="psI", bufs=1, space=bass.MemorySpace.PSUM) as psI,
        ):
            # ---- input DMAs on the two HWDGE queues, ordered by first use:
            #   sync:   locf (heads the mask chain), pT k0, pT k1
            #   scalar: cg, wb_a (We+eTW -> inj_pre), wb_b (Wp)
            locf_sb = small.tile([NP, 6], FP)
            nc.sync.dma_start(locf_sb[:], locf[:])
            cg_sb = small.tile([NP, CG], BF)
            nc.scalar.dma_start(cg_sb[:], cg[:])
            wb_sb = big.tile([128, WB], BF)
            nc.scalar.dma_start(wb_sb[:, 0:WB_A], wb[:, 0:WB_A])
            pT_sb = big.tile([128, 2 * HW], BF)
            nc.sync.dma_start(pT_sb[:, 0:HW], pT[:, 0:HW])  # k0
            nc.scalar.dma_start(wb_sb[:, WB_A:WB], wb[:, WB_A:WB])
            nc.sync.dma_start(pT_sb[:, HW : 2 * HW], pT[:, HW : 2 * HW])  # k1

            We_sb = [wb_sb[:, O * k : O * (k + 1)] for k in range(2)]
            eTW_sb = [
                wb_sb[:, 2 * O + 64 * k : 2 * O + 64 * (k + 1)] for k in range(2)
            ]
            Wp_sb = [wb_sb[:, WB_A + O * k : WB_A + O * (k + 1)] for k in range(2)]
            glo_b = cg_sb[:, 0:32]
            ghi_b = cg_sb[:, G_GH : G_GH + 32]
            yglo_b = cg_sb[:, G_YL : G_YL + N]
            yghi_b = cg_sb[:, G_YH : G_YH + N]
            W_s = cg_sb[:, G_WS : G_WS + 64]
            W_r = cg_sb[:, G_WR : G_WR + 64]

            # ---- row/col interval masks (bf16 0/1) straight from raw locs;
            # rowm2 rows 0:16 test y=0..15 (h0), rows 32:48 y=16..31 (h1)
            colm = small.tile([NP, 32], BF, name="colm")
            rowm2 = small.tile([NP, N], BF, name="rowm2")
            tmp_x = small.tile([NP, 32], BF, name="tmp_x")
            tmp_y = small.tile([NP, N], BF, name="tmp_y")
            nc.vector.tensor_scalar(
                tmp_x[:], ghi_b, locf_sb[:, 3:4], None, op0=AluOpType.is_lt
            )
            nc.vector.scalar_tensor_tensor(
                colm[:], glo_b, locf_sb[:, 1:2], tmp_x[:],
                op0=AluOpType.is_ge, op1=AluOpType.mult,
            )
            nc.vector.tensor_scalar(
                tmp_y[:], yghi_b, locf_sb[:, 2:3], None, op0=AluOpType.is_lt
            )
            nc.vector.scalar_tensor_tensor(
                rowm2[:], yglo_b, locf_sb[:, 0:1], tmp_y[:],
                op0=AluOpType.is_ge, op1=AluOpType.mult,
            )

            # ---- fused chain tiles: h=0 rows 0:16, h=1 rows 32:48
            mask = small.tile([NP, 512], BF, name="mask")
            ind = small.tile([NP, 512], BF, name="ind")
            maskN = small.tile([NP, 512], BF, name="maskN")
            psumS = psS.tile([64, 512], FP, name="psS")
            psumR = psS.tile([64, 512], FP, name="psR")
            psumI = psI.tile([64, O], FP, name="psI")
            psum = [[psT.tile([128, 512], FP, name=f"ps{h}{oc}") for oc in range(2)]
                    for h in range(2)]

            def mask_op():  # one fused DVE op; rows 16:32 harmless junk
                nc.vector.tensor_tensor(
                    _ap(mask[:, 0:512], [[W, N], [1, W]]),
                    _ap(rowm2[:, 0:N], [[1, N], [0, W]]),
                    _ap(colm[:, :], [[0, N], [1, W]]),
                    op=AluOpType.mult,
                )

            def s_mm():  # out rows 0:16 = s(h0), 32:48 = s(h1), 16:32 = 0
                return nc.tensor.matmul(
                    psumS[:], W_s, mask[:], start=True, stop=True
                )

            def iseq_op():  # partition p: ind = (s == (p&31)+1); junk rows 0
                nc.vector.tensor_scalar(
                    ind[:], psumS[0:NP, :], locf_sb[:, 4:5], None,
                    op0=AluOpType.is_equal,
                )

            def ind_mm():  # recB = 1/s replicated per strip; junk rows 0
                return nc.tensor.matmul(
                    psumR[:], W_r, ind[:], start=True, stop=True
                )

            def maskN_op():
                # (mask + 1/15) * recB  -- the +1/15 carries the mean-emb row
                nc.vector.scalar_tensor_tensor(
                    maskN[:], mask[:], 1.0 / NOBJ, psumR[0:NP, :],
                    op0=AluOpType.add, op1=AluOpType.mult,
                )

            def inj_pre_mm(k):  # inj on rows 0:15 AND 32:47 via duplicated eT
                return nc.tensor.matmul(
                    psumI[:], eTW_sb[k][:], We_sb[k][:],
                    start=(k == 0), stop=(k == 1),
                )

            def proj_mm(h, oc, k):
                return nc.tensor.matmul(
                    psum[h][oc][:],
                    Wp_sb[k][:, 128 * oc : 128 * (oc + 1)],
                    pT_sb[:, HW * k + 512 * h : HW * k + 512 * (h + 1)],
                    start=(k == 0), stop=False,
                )

            def inj_mm(h, oc):
                p0 = P2 * h
                return nc.tensor.matmul(
                    psum[h][oc][:],
                    inj_sb[p0 : p0 + NOBJ, 128 * oc : 128 * (oc + 1)],
                    maskN[p0 : p0 + NOBJ, :],
                    start=False, stop=True,
                )

            # ---- emission order doubles as per-engine FIFO order and
            # MUST be topological (Tile tracks deps by trace order).
            mask_op()

            pe = []
            pe.append(inj_pre_mm(0))
            pe.append(inj_pre_mm(1))
            inj_sb = small.tile([NP, O], BF, name="inj")
            nc.scalar.activation(inj_sb[:], psumI[0:NP, :], AF.Copy)

            pe.append(proj_mm(0, 0, 0))
            pe.append(s_mm())
            iseq_op()
            pe.append(proj_mm(1, 0, 0))
            pe.append(proj_mm(0, 0, 1))
            pe.append(proj_mm(1, 0, 1))
            pe.append(ind_mm())
            maskN_op()
            pe.append(proj_mm(0, 1, 0))
            pe.append(inj_mm(0, 0))
            pe.append(inj_mm(1, 0))

            # pair 1 evacuates on ACT+DVE and streams out while PE continues
            o_sb = outp.tile([128, 2 * HW], BF, name="osb")
            nc.scalar.activation(o_sb[:, 0:512], psum[0][0][:], AF.Copy)
            nc.vector.tensor_copy(o_sb[:, 512:1024], psum[1][0][:])
            nc.scalar.dma_start(out1[:], o_sb[:, 0:1024])

            pe.append(proj_mm(0, 1, 1))
            pe.append(proj_mm(1, 1, 0))
            pe.append(proj_mm(1, 1, 1))
            pe.append(inj_mm(0, 1))
            pe.append(inj_mm(1, 1))
            for a, b in zip(pe, pe[1:]):
                tile.add_dep_helper(b.ins, a.ins, sync=False, reason="PE order")

            nc.scalar.activation(o_sb[:, 1024:1536], psum[0][1][:], AF.Copy)
            nc.vector.tensor_copy(o_sb[:, 1536:2048], psum[1][1][:])
            nc.sync.dma_start(out2[:], o_sb[:, 1024:2048])

    nc.compile()
    return nc


def make_in_maps(inputs):
    import ml_dtypes

    bf16 = ml_dtypes.bfloat16
    patches = np.asarray(inputs["patches"], dtype=np.float32)
    embs = np.asarray(inputs["embs"], dtype=np.float32)
    locations = np.asarray(inputs["locations"], dtype=np.int32)
    Wp = np.asarray(inputs["Wp"], dtype=np.float32)
    We = np.asarray(inputs["We"], dtype=np.float32)
    img_box = np.array([[0, 0, H, W]], dtype=np.int32)

    # const gadget: parity grids, block-ones W_s, block-1/(p+1) W_r
    cg = np.zeros((NP, CG), dtype=np.float32)
    c = np.arange(32)
    cg[:, 0:32] = (c | 1)[None, :]
    cg[:, G_GH : G_GH + 32] = ((c & ~1) - 1)[None, :]
    y = np.arange(N)
    ys = np.zeros((NP, N), dtype=np.int64)
    ys[:] = y[None, :]
    ys[P2:] += N
    cg[:, G_YL : G_YL + N] = ys | 1
    cg[:, G_YH : G_YH + N] = (ys & ~1) - 1
    cg[0:N, G_WS : G_WS + N] = 1.0
    cg[P2 : P2 + N, G_WS + P2 : G_WS + P2 + N] = 1.0
    kv = (np.arange(NP) & 31) + 1
    wn = 1.0 / kv.astype(np.float32)
    cg[0:N, G_WR : G_WR + N] = wn[0:N, None]
    cg[P2 : P2 + N, G_WR + P2 : G_WR + P2 + N] = wn[P2 : P2 + N, None]
    cg_b = np.ascontiguousarray(cg.astype(bf16))

    wb_common = np.zeros((128, WB), dtype=np.float32)
    wb_common[:, 0:O] = We[0:128]
    wb_common[:, O : 2 * O] = We[128:256]
    wb_common[:, WB_A : WB_A + O] = Wp[0:128]
    wb_common[:, WB_A + O : WB] = Wp[128:256]

    in_maps = []
    for b in range(B):
        eTb = embs[b].T  # [256, 15]
        wbb = wb_common.copy()
        for k in range(2):
            base = 2 * O + 64 * k
            blk = eTb[128 * k : 128 * (k + 1)]
            wbb[:, base : base + NOBJ] = blk
            wbb[:, base + P2 : base + P2 + NOBJ] = blk
        # locf: raw fp32 box coords + the per-partition is_eq constant
        loc16 = np.concatenate([locations[b], img_box], 0)  # [16, 4]
        lf = np.zeros((NP, 6), dtype=np.float32)
        lf[:, 0:4] = np.tile(loc16, (3, 1))
        lf[:, 4] = kv
        pTb = patches[b].reshape(HW, D).T  # [256, 1024]
        pT2 = np.concatenate([pTb[0:128], pTb[128:256]], axis=1)  # [128, 2048]
        in_maps.append(
            {
                "locf": np.ascontiguousarray(lf),
                "cg": cg_b,
                "wb": np.ascontiguousarray(wbb.astype(bf16)),
                "pT": np.ascontiguousarray(pT2.astype(bf16)),
            }
        )
    return in_maps


_NC = None


def _get_nc():
    global _NC
    if _NC is None:
        _NC = build_nc(debug=False)
    return _NC


def run(inputs, trace: bool = False, **kwargs):
    nc = _get_nc()
    res = bass_utils.run_bass_kernel_spmd(
        nc, make_in_maps(inputs), core_ids=list(range(B)), trace=trace, **kwargs
    )
    outs = []
    for b in range(B):
        a1 = np.asarray(res.results[b]["out1"]).astype(np.float32)  # [128,1024]
        a2 = np.asarray(res.results[b]["out2"]).astype(np.float32)
        outs.append(np.concatenate([a1.T, a2.T], axis=1))  # [1024, 256]
    full = np.stack(outs, axis=0)
    return np.ascontiguousarray(full).astype(np.float32), res


def kernel(**inputs) -> np.ndarray:
    full, _ = run(inputs, trace=False)
    return full


# revision 10
# speedup vs baseline: 1.0535x; 1.0535x over previous
"""Trainium2 Bass kernel for nn_KnowledgeFusion.

Math (b=8, H=W=32, d=o=256, n_obj=15):
  embs_aug = concat([embs, mean(embs)])                  [b,16,256]
  mask     = rasterized boxes (rounded to PATCH_SIZE=2)  [b,16,1024] in {0,1}
  proj     = patches @ Wp                                [b,1024,256]
  inj      = embs_aug @ We                               [b,16,256]
  s[hw]    = sum_n mask[n,hw]   (>=1: image box row)
  out      = proj + (mask^T @ inj) / s[:,None]           [b,1024,256]

The mean-emb row folds away: with inj_k = embs_k @ We (k<15),
  outT[o,hw] = Wp^T @ patchesT + inj^T @ ((mask + 1/15) * recB)
where recB = 1/s, s integer in 1..16, recovered exactly via a one-hot
(is_equal against a per-partition constant) collapsed by a tiny matmul
against weights 1/(p+1).  (DVE reciprocal measures ~6.5 cycles/elem,
so the one-hot trick stays.)

v5 layout: the pixel halves h=0/h=1 live on partition strips 0:16 and
32:48.  Because DVE cost scales with free-size only (not partitions)
and matmul cost with streamed columns only, the whole chain is FUSED
across both halves: ONE DVE op each for mask / is_eq / maskN on
[48,512], and ONE 512-column matmul each for s and recB using [48,64]
block weights (rows 16:31 flow zeros).  The strip split is kept so the
four bank-closing inj matmuls (weights on rows 0:15 vs 32:47) still
pair up concurrently on the PE array's row strips.

The PATCH_SIZE box rounding is folded into parity-precomputed grids
(grid_lo[c] = c|1 >= start  <=>  c >= start&~1;  grid_hi2[c] =
(c&~1)-1 < end  <=>  c < end + 2-(end%2)), so raw fp32 locations feed
the comparisons directly -- the Vector queue is just 4 small compare
ops, one fused mask op, is_eq and maskN.  All constants ship
pre-computed in a tiny bf16 `cg` tensor; locations (+ the is_eq
per-partition constant) ship as fp32 `locf`.  Each dma_start costs
~0.6-0.8us of sequencer issue time and completion semaphores lag the
data by ~1us, so inputs use 6 triggers ordered by first use and
outputs 2 contiguous [128,1024] tensors, one per PSUM bank pair; each
pair evacuates on ACT+DVE concurrently the moment its banks close,
overlapped with remaining PE work.  The PE stream is ordered to never
stall so its p-state stays warm.  Everything is bf16 (fp32 PSUM
accumulation); rel-err ~4.6e-3 against the 2e-2 gate.
"""

import sys

sys.path.insert(0, "/opt/trn_rl_repo")

import numpy as np

import concourse.bass as bass
import concourse.bacc as bacc
import concourse.mybir as mybir
from concourse import tile
from concourse import bass_utils
from concourse.alu_op_type import AluOpType

B, H, W, D = 8, 32, 32, 256
NOBJ, N = 15, 16
HW = H * W
O = 256
P2 = 32  # partition offset of the h=1 group
NP = 48
FP = mybir.dt.float32
BF = mybir.dt.bfloat16
AF = mybir.ActivationFunctionType

# weights blob columns (bf16): We0 We1 eTW0 eTW1 W_s W_r | Wp0 Wp1
C_WS = 4 * O + 2 * 64
C_WR = C_WS + 64
WB_A = C_WR + 64  # 768: first DMA chunk
WB = WB_A + 2 * O  # 1280 total (second DMA carries Wp)

# lcg columns (fp32): loc(4) kvec(1) pad(1) grid_lo grid_hi2 yglo yghi2
G_GL = 6
G_GH = G_GL + 32
G_YL = G_GH + 32
G_YH = G_YL + N
CG = G_YH + N  # 102


def _ap(ap, free_dims):
    """AP with explicit free-dim [step, count] pairs (step 0 = broadcast)."""
    return bass.AP(ap.tensor, ap.offset, ap.ap[:1] + free_dims)


def build_nc(debug: bool = False):
    nc = bacc.Bacc("TRN2", target_bir_lowering=False, debug=debug, num_devices=B)

    lcg = nc.dram_tensor("lcg", [NP, CG], FP, kind="ExternalInput")
    wb = nc.dram_tensor("wb", [128, WB], BF, kind="ExternalInput")
    pT = nc.dram_tensor("pT", [128, 2 * HW], BF, kind="ExternalInput")
    out1 = nc.dram_tensor("out1", [128, HW], BF, kind="ExternalOutput")
    out2 = nc.dram_tensor("out2", [128, HW], BF, kind="ExternalOutput")

    with tile.TileContext(nc) as tc:
        with (
            nc.allow_low_precision(reason="bf16 matmuls, fp32 PSUM accumulation"),
            tc.tile_pool(name="big", bufs=1) as big,
            tc.tile_pool(name="small", bufs=1) as small,
            tc.tile_pool(name="outp", bufs=1) as outp,
            tc.tile_pool(name="psT", bufs=1, space=bass.MemorySpace.PSUM) as psT,
            tc.tile_pool(name="psS", bufs=1, space=bass.MemorySpace.PSUM) as psS,
            tc.tile_pool(name="psI", bufs=1, space=bass.MemorySpace.PSUM) as psI,
        ):
            # ---- input DMAs on the two HWDGE queues, ordered by first use:
            #   sync:   lcg (heads the mask chain), pT k0, pT k1
            #   scalar: wb_a (We+eTW+W_s+W_r), wb_b (Wp)
            lcg_sb = small.tile([NP, CG], FP)
            nc.sync.dma_start(lcg_sb[:], lcg[:])
            wb_sb = big.tile([128, WB], BF)
            nc.scalar.dma_start(wb_sb[:, 0:WB_A], wb[:, 0:WB_A])
            pT_sb = big.tile([128, 2 * HW], BF)
            nc.sync.dma_start(pT_sb[:, 0:HW], pT[:, 0:HW])  # k0
            nc.scalar.dma_start(wb_sb[:, WB_A:WB], wb[:, WB_A:WB])
            nc.sync.dma_start(pT_sb[:, HW : 2 * HW], pT[:, HW : 2 * HW])  # k1

            We_sb = [wb_sb[:, O * k : O * (k + 1)] for k in range(2)]
            eTW_sb = [
                wb_sb[:, 2 * O + 64 * k : 2 * O + 64 * (k + 1)] for k in range(2)
            ]
            Wp_sb = [wb_sb[:, WB_A + O * k : WB_A + O * (k + 1)] for k in range(2)]
            locf_sb = lcg_sb
            glo_b = lcg_sb[:, G_GL : G_GL + 32]
            ghi_b = lcg_sb[:, G_GH : G_GH + 32]
            yglo_b = lcg_sb[:, G_YL : G_YL + N]
            yghi_b = lcg_sb[:, G_YH : G_YH + N]
            W_s = wb_sb[0:NP, C_WS : C_WS + 64]
            W_r = wb_sb[0:NP, C_WR : C_WR + 64]

            # ---- row/col interval masks (bf16 0/1) straight from raw locs;
            # rowm2 rows 0:16 test y=0..15 (h0), rows 32:48 y=16..31 (h1)
            colm = small.tile([NP, 32], BF, name="colm")
            rowm2 = small.tile([NP, N], BF, name="rowm2")
            tmp_x = small.tile([NP, 32], BF, name="tmp_x")
            tmp_y = small.tile([NP, N], BF, name="tmp_y")
            nc.vector.tensor_scalar(
                tmp_x[:], ghi_b, locf_sb[:, 3:4], None, op0=AluOpType.is_lt
            )
            nc.vector.scalar_tensor_tensor(
                colm[:], glo_b, locf_sb[:, 1:2], tmp_x[:],
                op0=AluOpType.is_ge, op1=AluOpType.mult,
            )
            nc.vector.tensor_scalar(
                tmp_y[:], yghi_b, locf_sb[:, 2:3], None, op0=AluOpType.is_lt
            )
            nc.vector.scalar_tensor_tensor(
                rowm2[:], yglo_b, locf_sb[:, 0:1], tmp_y[:],
                op0=AluOpType.is_ge, op1=AluOpType.mult,
            )

            # ---- fused chain tiles: h=0 rows 0:16, h=1 rows 32:48
            mask = small.tile([NP, 512], BF, name="mask")
            ind = small.tile([NP, 512], BF, name="ind")
            maskN = small.tile([NP, 512], BF, name="maskN")
            psumS = psS.tile([64, 512], FP, name="psS")
            psumR = psS.tile([64, 512], FP, name="psR")
            psumI = psI.tile([64, O], FP, name="psI")
            psum = [[psT.tile([128, 512], FP, name=f"ps{h}{oc}") for oc in range(2)]
                    for h in range(2)]

            def mask_op():  # one fused DVE op; rows 16:32 harmless junk
                nc.vector.tensor_tensor(
                    _ap(mask[:, 0:512], [[W, N], [1, W]]),
                    _ap(rowm2[:, 0:N], [[1, N], [0, W]]),
                    _ap(colm[:, :], [[0, N], [1, W]]),
                    op=AluOpType.mult,
                )

            def s_mm():  # out rows 0:16 = s(h0), 32:48 = s(h1), 16:32 = 0
                return nc.tensor.matmul(
                    psumS[:], W_s, mask[:], start=True, stop=True
                )

            def iseq_op():  # partition p: ind = (s == (p&31)+1); junk rows 0
                nc.vector.tensor_scalar(
                    ind[:], psumS[0:NP, :], locf_sb[:, 4:5], None,
                    op0=AluOpType.is_equal,
                )

            def ind_mm():  # recB = 1/s replicated per strip; junk rows 0
                return nc.tensor.matmul(
                    psumR[:], W_r, ind[:], start=True, stop=True
                )

            def maskN_op():
                # (mask + 1/15) * recB  -- the +1/15 carries the mean-emb row
                nc.vector.scalar_tensor_tensor(
                    maskN[:], mask[:], 1.0 / NOBJ, psumR[0:NP, :],
                    op0=AluOpType.add, op1=AluOpType.mult,
                )

            def inj_pre_mm(k):  # inj on rows 0:15 AND 32:47 via duplicated eT
                return nc.tensor.matmul(
                    psumI[:], eTW_sb[k][:], We_sb[k][:],
                    start=(k == 0), stop=(k == 1),
                )

            def proj_mm(h, oc, k):
                return nc.tensor.matmul(
                    psum[h][oc][:],
                    Wp_sb[k][:, 128 * oc : 128 * (oc + 1)],
                    pT_sb[:, HW * k + 512 * h : HW * k + 512 * (h + 1)],
                    start=(k == 0), stop=False,
                )

            def inj_mm(h, oc):
                p0 = P2 * h
                return nc.tensor.matmul(
                    psum[h][oc][:],
                    inj_sb[p0 : p0 + NOBJ, 128 * oc : 128 * (oc + 1)],
                    maskN[p0 : p0 + NOBJ, :],
                    start=False, stop=True,
                )

            # ---- emission order doubles as per-engine FIFO order and
            # MUST be topological (Tile tracks deps by trace order).
            mask_op()

            pe = []
            pe.append(inj_pre_mm(0))
            pe.append(inj_pre_mm(1))
            inj_sb = small.tile([NP, O], BF, name="inj")
            nc.scalar.activation(inj_sb[:], psumI[0:NP, :], AF.Copy)

            pe.append(proj_mm(0, 0, 0))
            pe.append(s_mm())
            iseq_op()
            pe.append(proj_mm(1, 0, 0))
            pe.append(proj_mm(0, 0, 1))
            pe.append(proj_mm(1, 0, 1))
            pe.append(ind_mm())
            maskN_op()
            pe.append(proj_mm(0, 1, 0))
            pe.append(proj_mm(0, 1, 1))
            pe.append(inj_mm(0, 0))
            pe.append(inj_mm(1, 0))

            # pair 1 evacuates on ACT+DVE and streams out while PE continues
            o_sb = outp.tile([128, 2 * HW], BF, name="osb")
            nc.scalar.activation(o_sb[:, 0:512], psum[0][0][:], AF.Copy)
            nc.vector.tensor_copy(o_sb[:, 512:1024], psum[1][0][:])
            nc.scalar.dma_start(out1[:], o_sb[:, 0:1024])

            pe.append(proj_mm(1, 1, 0))
            pe.append(proj_mm(1, 1, 1))
            pe.append(inj_mm(0, 1))
            pe.append(inj_mm(1, 1))
            for a, b in zip(pe, pe[1:]):
                tile.add_dep_helper(b.ins, a.ins, sync=False, reason="PE order")

            nc.scalar.activation(o_sb[:, 1024:1536], psum[0][1][:], AF.Copy)
            nc.vector.tensor_copy(o_sb[:, 1536:2048], psum[1][1][:])
            nc.sync.dma_start(out2[:], o_sb[:, 1024:2048])

    nc.compile()
    return nc


def make_in_maps(inputs):
    import ml_dtypes

    bf16 = ml_dtypes.bfloat16
    patches = np.asarray(inputs["patches"], dtype=np.float32)
    embs = np.asarray(inputs["embs"], dtype=np.float32)
    locations = np.asarray(inputs["locations"], dtype=np.int32)
    Wp = np.asarray(inputs["Wp"], dtype=np.float32)
    We = np.asarray(inputs["We"], dtype=np.float32)
    img_box = np.array([[0, 0, H, W]], dtype=np.int32)

    # lcg: parity grids (+ per-batch loc/kvec filled below)
    lcg0 = np.zeros((NP, CG), dtype=np.float32)
    c = np.arange(32)
    lcg0[:, G_GL : G_GL + 32] = (c | 1)[None, :]
    lcg0[:, G_GH : G_GH + 32] = ((c & ~1) - 1)[None, :]
    y = np.arange(N)
    ys = np.zeros((NP, N), dtype=np.int64)
    ys[:] = y[None, :]
    ys[P2:] += N
    lcg0[:, G_YL : G_YL + N] = ys | 1
    lcg0[:, G_YH : G_YH + N] = (ys & ~1) - 1
    kv = (np.arange(NP) & 31) + 1
    wn = 1.0 / kv.astype(np.float32)

    wb_common = np.zeros((128, WB), dtype=np.float32)
    wb_common[:, 0:O] = We[0:128]
    wb_common[:, O : 2 * O] = We[128:256]
    wb_common[0:N, C_WS : C_WS + N] = 1.0
    wb_common[P2 : P2 + N, C_WS + P2 : C_WS + P2 + N] = 1.0
    wb_common[0:N, C_WR : C_WR + N] = wn[0:N, None]
    wb_common[P2 : P2 + N, C_WR + P2 : C_WR + P2 + N] = wn[P2 : P2 + N, None]
    wb_common[:, WB_A : WB_A + O] = Wp[0:128]
    wb_common[:, WB_A + O : WB] = Wp[128:256]

    in_maps = []
    for b in range(B):
        eTb = embs[b].T  # [256, 15]
        wbb = wb_common.copy()
        for k in range(2):
            base = 2 * O + 64 * k
            blk = eTb[128 * k : 128 * (k + 1)]
            wbb[:, base : base + NOBJ] = blk
            wbb[:, base + P2 : base + P2 + NOBJ] = blk
        # lcg: raw fp32 box coords + is_eq constant + parity grids
        loc16 = np.concatenate([locations[b], img_box], 0)  # [16, 4]
        lf = lcg0.copy()
        lf[:, 0:4] = np.tile(loc16, (3, 1))
        lf[:, 4] = kv
        pTb = patches[b].reshape(HW, D).T  # [256, 1024]
        pT2 = np.concatenate([pTb[0:128], pTb[128:256]], axis=1)  # [128, 2048]
        in_maps.append(
            {
                "lcg": np.ascontiguousarray(lf),
                "wb": np.ascontiguousarray(wbb.astype(bf16)),
                "pT": np.ascontiguousarray(pT2.astype(bf16)),
            }
        )
    return in_maps


_NC = None


def _get_nc():
    global _NC
    if _NC is None:
        _NC = build_nc(debug=False)
    return _NC


def run(inputs, trace: bool = False, **kwargs):
    nc = _get_nc()
    res = bass_utils.run_bass_kernel_spmd(
        nc, make_in_maps(inputs), core_ids=list(range(B)), trace=trace, **kwargs
    )
    outs = []
    for b in range(B):
        a1 = np.asarray(res.results[b]["out1"]).astype(np.float32)  # [128,1024]
        a2 = np.asarray(res.results[b]["out2"]).astype(np.float32)
        outs.append(np.concatenate([a1.T, a2.T], axis=1))  # [1024, 256]
    full = np.stack(outs, axis=0)
    return np.ascontiguousarray(full).astype(np.float32), res


def kernel(**inputs) -> np.ndarray:
    full, _ = run(inputs, trace=False)
    return full


# revision 11
# speedup vs baseline: 1.0561x; 1.0025x over previous
"""Trainium2 Bass kernel for nn_KnowledgeFusion.

Math (b=8, H=W=32, d=o=256, n_obj=15):
  embs_aug = concat([embs, mean(embs)])                  [b,16,256]
  mask     = rasterized boxes (rounded to PATCH_SIZE=2)  [b,16,1024] in {0,1}
  proj     = patches @ Wp                                [b,1024,256]
  inj      = embs_aug @ We                               [b,16,256]
  s[hw]    = sum_n mask[n,hw]   (>=1: image box row)
  out      = proj + (mask^T @ inj) / s[:,None]           [b,1024,256]

The mean-emb row folds away: with inj_k = embs_k @ We (k<15),
  outT[o,hw] = Wp^T @ patchesT + inj^T @ ((mask + 1/15) * recB)
where recB = 1/s, s integer in 1..16, recovered exactly via a one-hot
(is_equal against a per-partition constant) collapsed by a tiny matmul
against weights 1/(p+1).  (DVE reciprocal measures ~6.5 cycles/elem,
so the one-hot trick stays.)

v5 layout: the pixel halves h=0/h=1 live on partition strips 0:16 and
32:48.  Because DVE cost scales with free-size only (not partitions)
and matmul cost with streamed columns only, the whole chain is FUSED
across both halves: ONE DVE op each for mask / is_eq / maskN on
[48,512], and ONE 512-column matmul each for s and recB using [48,64]
block weights (rows 16:31 flow zeros).  The strip split is kept so the
four bank-closing inj matmuls (weights on rows 0:15 vs 32:47) still
pair up concurrently on the PE array's row strips.

The PATCH_SIZE box rounding is folded into parity-precomputed grids
(grid_lo[c] = c|1 >= start  <=>  c >= start&~1;  grid_hi2[c] =
(c&~1)-1 < end  <=>  c < end + 2-(end%2)), so raw fp32 locations feed
the comparisons directly -- the Vector queue is just 4 small compare
ops, one fused mask op, is_eq and maskN.  All constants ship
pre-computed in a tiny bf16 `cg` tensor; locations (+ the is_eq
per-partition constant) ship as fp32 `locf`.  Each dma_start costs
~0.6-0.8us of sequencer issue time and completion semaphores lag the
data by ~1us, so inputs use 6 triggers ordered by first use and
outputs 2 contiguous [128,1024] tensors, one per PSUM bank pair; each
pair evacuates on ACT+DVE concurrently the moment its banks close,
overlapped with remaining PE work.  The PE stream is ordered to never
stall so its p-state stays warm.  Everything is bf16 (fp32 PSUM
accumulation); rel-err ~4.6e-3 against the 2e-2 gate.
"""

import sys

sys.path.insert(0, "/opt/trn_rl_repo")

import numpy as np

import concourse.bass as bass
import concourse.bacc as bacc
import concourse.mybir as mybir
from concourse import tile
from concourse import bass_utils
from concourse.alu_op_type import AluOpType

B, H, W, D = 8, 32, 32, 256
NOBJ, N = 15, 16
HW = H * W
O = 256
P2 = 32  # partition offset of the h=1 group
NP = 48
FP = mybir.dt.float32
BF = mybir.dt.bfloat16
AF = mybir.ActivationFunctionType

# weights blob columns (bf16): We0 We1 eTW0 eTW1 W_s W_r | Wp0 Wp1
C_WS = 4 * O + 2 * 64
C_WR = C_WS + 64
WB_A = C_WR + 64  # 768: first DMA chunk
WB = WB_A + 2 * O  # 1280 total (second DMA carries Wp)

# lcg columns (fp32): loc(4) kvec(1) pad(1) grid_lo grid_hi2 yglo yghi2
G_GL = 6
G_GH = G_GL + 32
G_YL = G_GH + 32
G_YH = G_YL + N
CG = G_YH + N  # 102


def _ap(ap, free_dims):
    """AP with explicit free-dim [step, count] pairs (step 0 = broadcast)."""
    return bass.AP(ap.tensor, ap.offset, ap.ap[:1] + free_dims)


def build_nc(debug: bool = False):
    nc = bacc.Bacc("TRN2", target_bir_lowering=False, debug=debug, num_devices=B)

    lcg = nc.dram_tensor("lcg", [NP, CG], FP, kind="ExternalInput")
    wb = nc.dram_tensor("wb", [128, WB], BF, kind="ExternalInput")
    pT = nc.dram_tensor("pT", [128, 2 * HW], BF, kind="ExternalInput")
    out1 = nc.dram_tensor("out1", [128, HW], BF, kind="ExternalOutput")
    out2a = nc.dram_tensor("out2a", [128, 512], BF, kind="ExternalOutput")
    out2b = nc.dram_tensor("out2b", [128, 512], BF, kind="ExternalOutput")

    with tile.TileContext(nc) as tc:
        with (
            nc.allow_low_precision(reason="bf16 matmuls, fp32 PSUM accumulation"),
            tc.tile_pool(name="big", bufs=1) as big,
            tc.tile_pool(name="small", bufs=1) as small,
            tc.tile_pool(name="outp", bufs=1) as outp,
            tc.tile_pool(name="psT", bufs=1, space=bass.MemorySpace.PSUM) as psT,
            tc.tile_pool(name="psS", bufs=1, space=bass.MemorySpace.PSUM) as psS,
            tc.tile_pool(name="psI", bufs=1, space=bass.MemorySpace.PSUM) as psI,
        ):
            # ---- input DMAs on the two HWDGE queues, ordered by first use:
            #   sync:   lcg (heads the mask chain), pT k0, pT k1
            #   scalar: wb_a (We+eTW+W_s+W_r), wb_b (Wp)
            lcg_sb = small.tile([NP, CG], FP)
            nc.sync.dma_start(lcg_sb[:], lcg[:])
            wb_sb = big.tile([128, WB], BF)
            nc.scalar.dma_start(wb_sb[:, 0:WB_A], wb[:, 0:WB_A])
            pT_sb = big.tile([128, 2 * HW], BF)
            nc.sync.dma_start(pT_sb[:, 0:HW], pT[:, 0:HW])  # k0
            nc.scalar.dma_start(wb_sb[:, WB_A:WB], wb[:, WB_A:WB])
            nc.sync.dma_start(pT_sb[:, HW : 2 * HW], pT[:, HW : 2 * HW])  # k1

            We_sb = [wb_sb[:, O * k : O * (k + 1)] for k in range(2)]
            eTW_sb = [
                wb_sb[:, 2 * O + 64 * k : 2 * O + 64 * (k + 1)] for k in range(2)
            ]
            Wp_sb = [wb_sb[:, WB_A + O * k : WB_A + O * (k + 1)] for k in range(2)]
            locf_sb = lcg_sb
            glo_b = lcg_sb[:, G_GL : G_GL + 32]
            ghi_b = lcg_sb[:, G_GH : G_GH + 32]
            yglo_b = lcg_sb[:, G_YL : G_YL + N]
            yghi_b = lcg_sb[:, G_YH : G_YH + N]
            W_s = wb_sb[0:NP, C_WS : C_WS + 64]
            W_r = wb_sb[0:NP, C_WR : C_WR + 64]

            # ---- row/col interval masks (bf16 0/1) straight from raw locs;
            # rowm2 rows 0:16 test y=0..15 (h0), rows 32:48 y=16..31 (h1)
            colm = small.tile([NP, 32], BF, name="colm")
            rowm2 = small.tile([NP, N], BF, name="rowm2")
            tmp_x = small.tile([NP, 32], BF, name="tmp_x")
            tmp_y = small.tile([NP, N], BF, name="tmp_y")
            nc.vector.tensor_scalar(
                tmp_x[:], ghi_b, locf_sb[:, 3:4], None, op0=AluOpType.is_lt
            )
            nc.vector.scalar_tensor_tensor(
                colm[:], glo_b, locf_sb[:, 1:2], tmp_x[:],
                op0=AluOpType.is_ge, op1=AluOpType.mult,
            )
            nc.vector.tensor_scalar(
                tmp_y[:], yghi_b, locf_sb[:, 2:3], None, op0=AluOpType.is_lt
            )
            nc.vector.scalar_tensor_tensor(
                rowm2[:], yglo_b, locf_sb[:, 0:1], tmp_y[:],
                op0=AluOpType.is_ge, op1=AluOpType.mult,
            )

            # ---- fused chain tiles: h=0 rows 0:16, h=1 rows 32:48
            mask = small.tile([NP, 512], BF, name="mask")
            ind = small.tile([NP, 512], BF, name="ind")
            maskN = small.tile([NP, 512], BF, name="maskN")
            psumS = psS.tile([64, 512], FP, name="psS")
            psumR = psS.tile([64, 512], FP, name="psR")
            psumI = psI.tile([64, O], FP, name="psI")
            psum = [[psT.tile([128, 512], FP, name=f"ps{h}{oc}") for oc in range(2)]
                    for h in range(2)]

            def mask_op():  # one fused DVE op; rows 16:32 harmless junk
                nc.vector.tensor_tensor(
                    _ap(mask[:, 0:512], [[W, N], [1, W]]),
                    _ap(rowm2[:, 0:N], [[1, N], [0, W]]),
                    _ap(colm[:, :], [[0, N], [1, W]]),
                    op=AluOpType.mult,
                )

            def s_mm():  # out rows 0:16 = s(h0), 32:48 = s(h1), 16:32 = 0
                return nc.tensor.matmul(
                    psumS[:], W_s, mask[:], start=True, stop=True
                )

            def iseq_op():  # partition p: ind = (s == (p&31)+1); junk rows 0
                nc.vector.tensor_scalar(
                    ind[:], psumS[0:NP, :], locf_sb[:, 4:5], None,
                    op0=AluOpType.is_equal,
                )

            def ind_mm():  # recB = 1/s replicated per strip; junk rows 0
                return nc.tensor.matmul(
                    psumR[:], W_r, ind[:], start=True, stop=True
                )

            def maskN_op():
                # (mask + 1/15) * recB  -- the +1/15 carries the mean-emb row
                nc.vector.scalar_tensor_tensor(
                    maskN[:], mask[:], 1.0 / NOBJ, psumR[0:NP, :],
                    op0=AluOpType.add, op1=AluOpType.mult,
                )

            def inj_pre_mm(k):  # inj on rows 0:15 AND 32:47 via duplicated eT
                return nc.tensor.matmul(
                    psumI[:], eTW_sb[k][:], We_sb[k][:],
                    start=(k == 0), stop=(k == 1),
                )

            def proj_mm(h, oc, k):
                return nc.tensor.matmul(
                    psum[h][oc][:],
                    Wp_sb[k][:, 128 * oc : 128 * (oc + 1)],
                    pT_sb[:, HW * k + 512 * h : HW * k + 512 * (h + 1)],
                    start=(k == 0), stop=False,
                )

            def inj_mm(h, oc):
                p0 = P2 * h
                return nc.tensor.matmul(
                    psum[h][oc][:],
                    inj_sb[p0 : p0 + NOBJ, 128 * oc : 128 * (oc + 1)],
                    maskN[p0 : p0 + NOBJ, :],
                    start=False, stop=True,
                )

            # ---- emission order doubles as per-engine FIFO order and
            # MUST be topological (Tile tracks deps by trace order).
            mask_op()

            pe = []
            pe.append(inj_pre_mm(0))
            pe.append(inj_pre_mm(1))
            inj_sb = small.tile([NP, O], BF, name="inj")
            nc.scalar.activation(inj_sb[:], psumI[0:NP, :], AF.Copy)

            pe.append(s_mm())
            iseq_op()
            pe.append(proj_mm(0, 0, 0))
            pe.append(proj_mm(1, 0, 0))
            pe.append(proj_mm(0, 0, 1))
            pe.append(ind_mm())
            maskN_op()
            pe.append(proj_mm(1, 0, 1))
            pe.append(proj_mm(0, 1, 0))
            pe.append(proj_mm(0, 1, 1))
            pe.append(inj_mm(0, 0))
            pe.append(inj_mm(1, 0))

            # pair 1 evacuates on ACT+DVE and streams out while PE continues
            o_sb = outp.tile([128, 2 * HW], BF, name="osb")
            nc.scalar.activation(o_sb[:, 0:512], psum[0][0][:], AF.Copy)
            nc.vector.tensor_copy(o_sb[:, 512:1024], psum[1][0][:])
            nc.scalar.dma_start(out1[:], o_sb[:, 0:1024])

            pe.append(proj_mm(1, 1, 0))
            pe.append(proj_mm(1, 1, 1))
            pe.append(inj_mm(0, 1))
            pe.append(inj_mm(1, 1))
            for a, b in zip(pe, pe[1:]):
                tile.add_dep_helper(b.ins, a.ins, sync=False, reason="PE order")

            nc.scalar.activation(o_sb[:, 1024:1536], psum[0][1][:], AF.Copy)
            nc.vector.tensor_copy(o_sb[:, 1536:2048], psum[1][1][:])
            nc.scalar.dma_start(out2a[:], o_sb[:, 1024:1536])
            nc.sync.dma_start(out2b[:], o_sb[:, 1536:2048])

    nc.compile()
    return nc


def make_in_maps(inputs):
    import ml_dtypes

    bf16 = ml_dtypes.bfloat16
    patches = np.asarray(inputs["patches"], dtype=np.float32)
    embs = np.asarray(inputs["embs"], dtype=np.float32)
    locations = np.asarray(inputs["locations"], dtype=np.int32)
    Wp = np.asarray(inputs["Wp"], dtype=np.float32)
    We = np.asarray(inputs["We"], dtype=np.float32)
    img_box = np.array([[0, 0, H, W]], dtype=np.int32)

    # lcg: parity grids (+ per-batch loc/kvec filled below)
    lcg0 = np.zeros((NP, CG), dtype=np.float32)
    c = np.arange(32)
    lcg0[:, G_GL : G_GL + 32] = (c | 1)[None, :]
    lcg0[:, G_GH : G_GH + 32] = ((c & ~1) - 1)[None, :]
    y = np.arange(N)
    ys = np.zeros((NP, N), dtype=np.int64)
    ys[:] = y[None, :]
    ys[P2:] += N
    lcg0[:, G_YL : G_YL + N] = ys | 1
    lcg0[:, G_YH : G_YH + N] = (ys & ~1) - 1
    kv = (np.arange(NP) & 31) + 1
    wn = 1.0 / kv.astype(np.float32)

    wb_common = np.zeros((128, WB), dtype=np.float32)
    wb_common[:, 0:O] = We[0:128]
    wb_common[:, O : 2 * O] = We[128:256]
    wb_common[0:N, C_WS : C_WS + N] = 1.0
    wb_common[P2 : P2 + N, C_WS + P2 : C_WS + P2 + N] = 1.0
    wb_common[0:N, C_WR : C_WR + N] = wn[0:N, None]
    wb_common[P2 : P2 + N, C_WR + P2 : C_WR + P2 + N] = wn[P2 : P2 + N, None]
    wb_common[:, WB_A : WB_A + O] = Wp[0:128]
    wb_common[:, WB_A + O : WB] = Wp[128:256]

    in_maps = []
    for b in range(B):
        eTb = embs[b].T  # [256, 15]
        wbb = wb_common.copy()
        for k in range(2):
            base = 2 * O + 64 * k
            blk = eTb[128 * k : 128 * (k + 1)]
            wbb[:, base : base + NOBJ] = blk
            wbb[:, base + P2 : base + P2 + NOBJ] = blk
        # lcg: raw fp32 box coords + is_eq constant + parity grids
        loc16 = np.concatenate([locations[b], img_box], 0)  # [16, 4]
        lf = lcg0.copy()
        lf[:, 0:4] = np.tile(loc16, (3, 1))
        lf[:, 4] = kv
        pTb = patches[b].reshape(HW, D).T  # [256, 1024]
        pT2 = np.concatenate([pTb[0:128], pTb[128:256]], axis=1)  # [128, 2048]
        in_maps.append(
            {
                "lcg": np.ascontiguousarray(lf),
                "wb": np.ascontiguousarray(wbb.astype(bf16)),
                "pT": np.ascontiguousarray(pT2.astype(bf16)),
            }
        )
    return in_maps


_NC = None


def _get_nc():
    global _NC
    if _NC is None:
        _NC = build_nc(debug=False)
    return _NC


def run(inputs, trace: bool = False, **kwargs):
    nc = _get_nc()
    res = bass_utils.run_bass_kernel_spmd(
        nc, make_in_maps(inputs), core_ids=list(range(B)), trace=trace, **kwargs
    )
    outs = []
    for b in range(B):
        a1 = np.asarray(res.results[b]["out1"]).astype(np.float32)  # [128,1024]
        a2 = np.concatenate(
            [np.asarray(res.results[b]["out2a"]),
             np.asarray(res.results[b]["out2b"])], axis=1).astype(np.float32)
        outs.append(np.concatenate([a1.T, a2.T], axis=1))  # [1024, 256]
    full = np.stack(outs, axis=0)
    return np.ascontiguousarray(full).astype(np.float32), res


def kernel(**inputs) -> np.ndarray:
    full, _ = run(inputs, trace=False)
    return full
